# revision 6
# baseline (speedup 1.0000x reference)
"""GQA kernel for Trainium2, 8 NeuronCores.

Problem: x[2,2048,2048] -> GQA(16 heads, 4 kv groups, dk=128) -> out[2,2048,2048]

Sharding: core c handles (batch b = c//4, kv-group g = c%4), i.e. the 4 query
heads of one group on one batch. Zero replication of FLOPs across cores:
per-core work = Qproj(4 heads) + K/Vproj(1 group) + attention(4 heads) +
row-slice of the O projection. Host sums the 4 per-group partial outputs
per batch (the row-parallel O-proj reduction) and adds bo.

On-core dataflow (all matmuls contract over the partition dim):
  xT [D,S] (host-transposed)  --PE-->  QT [dk,S] per head, KT [dk,S], VT [dk,S]
  scoresT[sk,sq] = KT_chunk.T @ QT      (f32r, full-rate)
  attnT = exp(scoresT / sqrt(dk))       (ACT, bf16 out)
  attn@[V|1] via lhsT=attnT chunks      (bf16; extra ones-column of V gives the
                                         softmax denominator for free)
  normalize rows (DVE), PE-transpose back to [dk,sq], O-proj vs Wo rows (f32r)
"""

import math

import numpy as np

import concourse.bass as bass
import concourse.mybir as mybir
import concourse.tile as tile
from concourse import bacc
from concourse.bass_utils import run_bass_kernel_spmd
from concourse.masks import make_identity

F32 = mybir.dt.float32
F32R = mybir.dt.float32r
BF16 = mybir.dt.bfloat16

D = 2048          # d_model
S = 2048          # seq len
DK = 128          # head dim
HPG = 4           # heads per kv group
QCOLS = HPG * DK  # 512 q columns per core
N_CORES = 8
SCALE = 1.0 / math.sqrt(DK)

SJ = 256                    # seq chunk (free dim of proj/scores matmuls)
NJ = S // SJ                # 8 chunks
NSK = S // 128              # 16 key chunks
ND = D // 128               # 16 d_model chunks


def _r(ap):
    return ap.bitcast(F32R)


def build_program():
    nc = bacc.Bacc("TRN2", target_bir_lowering=False, debug=False,
                   num_devices=N_CORES)

    xt = nc.dram_tensor("xt", [D, S], F32, kind="ExternalInput").ap()
    wq = nc.dram_tensor("wq", [D, QCOLS], F32, kind="ExternalInput").ap()
    wk = nc.dram_tensor("wk", [D, DK], F32, kind="ExternalInput").ap()
    wv = nc.dram_tensor("wv", [D, DK], F32, kind="ExternalInput").ap()
    wo = nc.dram_tensor("wo", [QCOLS, D], F32, kind="ExternalInput").ap()
    bq = nc.dram_tensor("bq", [QCOLS], F32, kind="ExternalInput").ap()
    bk = nc.dram_tensor("bk", [DK], F32, kind="ExternalInput").ap()
    bv = nc.dram_tensor("bv", [DK], F32, kind="ExternalInput").ap()
    out = nc.dram_tensor("out", [S, D], F32, kind="ExternalOutput").ap()

    with tile.TileContext(nc) as tc:
        with (
            tc.tile_pool(name="singles", bufs=1) as singles,
            tc.tile_pool(name="xp", bufs=3) as xpool,
            tc.tile_pool(name="attn", bufs=20) as attnpool,
            tc.tile_pool(name="aot", bufs=2) as aotpool,
            tc.tile_pool(name="osb", bufs=3) as outpool,
            tc.tile_pool(name="small", bufs=4) as smallpool,
            tc.tile_pool(name="psA", bufs=2, space="PSUM") as psA,
            tc.tile_pool(name="psAV", bufs=2, space="PSUM") as psAV,
            tc.tile_pool(name="psT", bufs=2, space="PSUM") as psT,
            tc.tile_pool(name="psO", bufs=2, space="PSUM") as psO,
        ):
            # ---- resident weights / biases ----
            wq_sb = singles.tile([128, ND, QCOLS], F32R)
            nc.sync.dma_start(out=wq_sb, in_=wq.rearrange("(c p) n -> p c n", p=128).bitcast(F32R))
            wk_sb = singles.tile([128, ND, DK], F32R)
            nc.sync.dma_start(out=wk_sb, in_=wk.rearrange("(c p) n -> p c n", p=128).bitcast(F32R))
            wv_sb = singles.tile([128, ND, DK], F32R)
            nc.sync.dma_start(out=wv_sb, in_=wv.rearrange("(c p) n -> p c n", p=128).bitcast(F32R))
            wo_sb = singles.tile([128, HPG, D], F32R)
            nc.sync.dma_start(out=wo_sb, in_=wo.rearrange("(h p) n -> p h n", p=128).bitcast(F32R))
            bq_sb = singles.tile([128, HPG], F32)
            nc.sync.dma_start(out=bq_sb, in_=bq.rearrange("(h p) -> p h", p=128))
            bk_sb = singles.tile([128, 1], F32)
            nc.sync.dma_start(out=bk_sb, in_=bk.unsqueeze(1))
            bv_sb = singles.tile([128, 1], F32)
            nc.sync.dma_start(out=bv_sb, in_=bv.unsqueeze(1))

            ident32 = singles.tile([128, 128], F32)
            make_identity(nc, ident32)
            ident16 = singles.tile([128, 128], BF16)
            make_identity(nc, ident16)

            qt_sb = singles.tile([128, HPG, S], F32R)    # QT per head [dk, S]
            kt_sb = singles.tile([128, S], F32R)         # KT [dk, S]
            vt_sb = singles.tile([128, S], BF16)        # VT [dk, S]
            vones = singles.tile([128, NSK, 132], BF16)  # [V | 1] per key chunk
            nc.vector.memset(vones[:, :, 128:129], 1.0)

            # ---- phase B: projections (stream xT chunks) ----
            for j in range(NJ):
                sl = bass.ts(j, SJ)
                xt_sb = xpool.tile([128, ND, SJ], F32R)
                nc.sync.dma_start(
                    out=xt_sb, in_=xt[:, sl].rearrange("(c p) s -> p c s", p=128).bitcast(F32R))
                for h in range(HPG):
                    pq = psA.tile([128, SJ], F32, tag="big")
                    for d in range(ND):
                        nc.tensor.matmul(
                            pq, lhsT=wq_sb[:, d, bass.ts(h, 128)],
                            rhs=xt_sb[:, d, :],
                            start=(d == 0), stop=(d == ND - 1))
                    nc.scalar.activation(
                        out=qt_sb[:, h, sl], in_=pq,
                        func=mybir.ActivationFunctionType.Identity,
                        bias=bq_sb[:, h:h + 1])
                pk = psA.tile([128, SJ], F32, tag="big")
                for d in range(ND):
                    nc.tensor.matmul(pk, lhsT=wk_sb[:, d, :],
                                     rhs=xt_sb[:, d, :],
                                     start=(d == 0), stop=(d == ND - 1))
                nc.scalar.activation(out=kt_sb[:, sl], in_=pk,
                                     func=mybir.ActivationFunctionType.Identity,
                                     bias=bk_sb)
                pv = psA.tile([128, SJ], F32, tag="big")
                for d in range(ND):
                    nc.tensor.matmul(pv, lhsT=wv_sb[:, d, :],
                                     rhs=xt_sb[:, d, :],
                                     start=(d == 0), stop=(d == ND - 1))
                nc.scalar.activation(out=vt_sb[:, sl], in_=pv,
                                     func=mybir.ActivationFunctionType.Identity,
                                     bias=bv_sb)

            # VT [dk,S] -> V [S,dk] chunks with a ones column appended
            for sk in range(NSK):
                pt = psT.tile([128, 128], BF16, tag="t")
                nc.tensor.transpose(pt, vt_sb[:, bass.ts(sk, 128)], ident16)
                nc.vector.tensor_copy(vones[:, sk, 0:128], pt)

            # ---- phase C: attention + O-projection, per 256-wide q block ----
            for j in range(NJ):
                sl = bass.ts(j, SJ)
                aot = aotpool.tile([128, HPG, SJ], F32R)  # attn-out.T [dk, sq]
                for h in range(HPG):
                    attns = []
                    for sk in range(NSK):
                        ps = psA.tile([128, SJ], F32, tag="big")
                        nc.tensor.matmul(ps, lhsT=kt_sb[:, bass.ts(sk, 128)],
                                         rhs=qt_sb[:, h, sl],
                                         start=True, stop=True)
                        a = attnpool.tile([128, SJ], BF16)
                        nc.scalar.activation(
                            out=a, in_=ps,
                            func=mybir.ActivationFunctionType.Exp, scale=SCALE)
                        attns.append(a)
                    for sub in range(SJ // 128):
                        pav = psAV.tile([128, 132], F32, tag="av")
                        for sk in range(NSK):
                            nc.tensor.matmul(
                                pav[:, 0:129],
                                lhsT=attns[sk][:, bass.ts(sub, 128)],
                                rhs=vones[:, sk, 0:129],
                                start=(sk == 0), stop=(sk == NSK - 1))
                        recip = smallpool.tile([128, 1], F32)
                        nc.vector.reciprocal(recip, pav[:, 128:129])
                        ao = smallpool.tile([128, 128], F32, tag="ao")
                        nc.vector.tensor_scalar_mul(ao, pav[:, 0:128], recip)
                        pt = psT.tile([128, 128], F32, tag="t")
                        nc.tensor.transpose(pt, ao, ident32)
                        nc.vector.tensor_copy(aot[:, h, bass.ts(sub, 128)], pt)
                # O projection for q rows [j*SJ, (j+1)*SJ)
                for sub in range(SJ // 128):
                    for dc in range(D // 512):
                        po = psO.tile([128, 512], F32, tag="o")
                        for h in range(HPG):
                            nc.tensor.matmul(
                                po, lhsT=aot[:, h, bass.ts(sub, 128)],
                                rhs=wo_sb[:, h, bass.ts(dc, 512)],
                                start=(h == 0), stop=(h == HPG - 1))
                        osb = outpool.tile([128, 512], F32)
                        nc.vector.tensor_copy(osb, po)
                        nc.sync.dma_start(
                            out=out[j * SJ + sub * 128: j * SJ + (sub + 1) * 128,
                                    bass.ts(dc, 512)],
                            in_=osb)

    nc.compile()
    return nc


_NC_CACHE = None


def _get_program():
    global _NC_CACHE
    if _NC_CACHE is None:
        _NC_CACHE = build_program()
    return _NC_CACHE


def kernel(x, Wq, bq, Wk, bk, Wv, bv, Wo, bo):
    x = np.asarray(x, np.float32)
    nc = _get_program()

    in_maps = []
    xts = [np.ascontiguousarray(x[b].T) for b in range(x.shape[0])]
    for c in range(N_CORES):
        b, g = divmod(c, HPG)
        in_maps.append({
            "xt": xts[b],
            "wq": np.ascontiguousarray(np.asarray(Wq, np.float32)[:, g * QCOLS:(g + 1) * QCOLS]),
            "wk": np.ascontiguousarray(np.asarray(Wk, np.float32)[:, g * DK:(g + 1) * DK]),
            "wv": np.ascontiguousarray(np.asarray(Wv, np.float32)[:, g * DK:(g + 1) * DK]),
            "wo": np.ascontiguousarray(np.asarray(Wo, np.float32)[g * QCOLS:(g + 1) * QCOLS, :]),
            "bq": np.ascontiguousarray(np.asarray(bq, np.float32)[g * QCOLS:(g + 1) * QCOLS]),
            "bk": np.ascontiguousarray(np.asarray(bk, np.float32)[g * DK:(g + 1) * DK]),
            "bv": np.ascontiguousarray(np.asarray(bv, np.float32)[g * DK:(g + 1) * DK]),
        })

    res = run_bass_kernel_spmd(nc, in_maps, core_ids=list(range(N_CORES))).results

    outv = np.zeros((x.shape[0], S, D), np.float32)
    for c in range(N_CORES):
        b = c // HPG
        outv[b] += res[c]["out"]
    outv += np.asarray(bo, np.float32)
    return outv
